# revision 8
# baseline (speedup 1.0000x reference)
"""Trainium2 Bass kernel for the 2-bit-DoReFa quantized BasicBlock.

  out = conv3x3(q(bn2(conv3x3(q(bn1(x)), Wq1))), Wq2) + x
  q(h) = round(3*clip(relu(h),0,1))/3,  Wq = DoReFa-2bit(w) in {-1,-1/3,1/3,1}

Sharding: data-parallel over batch, 4 images per NeuronCore x 8 cores;
conv weights and BN parameters replicated.

Per-core kernel design:
  * Quantized activations/weights are exact small integers when scaled by 3:
    a3 in {0..3}, w3 in {-3,-1,1,3}.  Both are exact in fp8e4, so each 3x3
    conv becomes 9 accumulating DoubleRow 128x(2x128) matmuls per output
    tile with *exact* integer accumulation in fp32 PSUM.  The 1/9 rescale
    folds into the next stage's BN scale / final residual add.
  * Round-to-nearest-even == (t + 2^23) - 2^23 in fp32, bit-matching
    jnp.round; the whole BN/relu/clip/round chain replicates the reference
    fp32 op sequence operation-for-operation on the DVE/ACT engines.
  * Conv reads a zero-padded 58-wide image in SBUF; pixel tiles of 464
    columns (8 rows) keep each matmul accumulation in one PSUM bank.
  * Software pipeline: stage1(i+1) | conv1(i) | conv2(i-1); stage-1 quant
    units are spread between conv1 tiles (avoids DVE head-of-line blocks),
    per-image input DMAs are serialized so the critical image gets full
    HBM bandwidth, and a PE warmup burst runs during the initial DMA
    window to bring the HAM clock gate to 8/8 before the first real matmul.
"""
import os
from contextlib import ExitStack

import numpy as np

import concourse.bacc as bacc
import concourse.tile as tile
from concourse import mybir
from concourse.bass_utils import run_bass_kernel_spmd

F32 = mybir.dt.float32
OP = mybir.AluOpType
MAGIC = 8388608.0  # 2**23

N_CORES = 8
N_IMG = 4
C = 256
H = W = 56
PW = W + 2
NPIX = H * W
RT = 8
NT = H // RT
TQ = RT * PW                                   # 464
NPAD = ((PW * (H + 2) + 2 + 15) // 16) * 16    # 3376
N_CHUNK = 4
CR = H // N_CHUNK
# Uneven stage-1 chunking: small leading chunks unlock the first conv tiles
# early; bigger trailing chunks keep per-op overhead low.
CHUNKS = [(0, 8), (8, 8), (16, 20), (36, 20)]
ACT_DT = mybir.dt.float8e4
N_WARMUP = 78

LAST_EXEC_NS = None          # set when BASS_TRACE=1
_CACHED = {}


def _build():
    nc = bacc.Bacc("TRN2", target_bir_lowering=False, debug=False)

    x_d = nc.dram_tensor("x", [N_IMG, C, H, W], F32, kind="ExternalInput")
    w1_d = nc.dram_tensor("w1t", [128, 4608], ACT_DT, kind="ExternalInput")
    w2_d = nc.dram_tensor("w2t", [128, 4608], ACT_DT, kind="ExternalInput")
    prm_d = nc.dram_tensor("prm", [128, 8], F32, kind="ExternalInput")
    out_d = nc.dram_tensor("out", [N_IMG, C, H, W], F32, kind="ExternalOutput")

    xr = x_d.ap().rearrange("n (b k) h w -> n k b (h w)", b=2)
    outr = out_d.ap().rearrange("n (b k) h w -> n k b (h w)", b=2)

    with tile.TileContext(nc) as tc, ExitStack() as ctx:
        wpool = ctx.enter_context(tc.tile_pool(name="wpool", bufs=1))
        xpool = ctx.enter_context(tc.tile_pool(name="xpool", bufs=4))
        aqpool = ctx.enter_context(tc.tile_pool(name="aqpool", bufs=1))
        t1pool = ctx.enter_context(tc.tile_pool(name="t1pool", bufs=3))
        t2pool = ctx.enter_context(tc.tile_pool(name="t2pool", bufs=6))
        pspool = ctx.enter_context(tc.tile_pool(name="pspool", bufs=7,
                                                space="PSUM"))

        prm = wpool.tile([128, 8], F32)
        nc.sync.dma_start(prm[:], prm_d.ap())
        w1_sb = wpool.tile([128, 4608], ACT_DT)
        nc.sync.dma_start(w1_sb[:], w1_d.ap())
        w2_sb = wpool.tile([128, 4608], ACT_DT)
        nc.sync.dma_start(w2_sb[:], w2_d.ap())

        # Fixed ping-pong padded activation buffers; borders zeroed once
        # (interior writes never touch them, so they stay zero on reuse).
        aq1s, aq2s = [], []
        for i in range(2):
            a1 = aqpool.tile([128, 2, NPAD], ACT_DT, name=f"aq1_{i}", tag=f"aq1_{i}")
            a2 = aqpool.tile([128, 2, NPAD], ACT_DT, name=f"aq2_{i}", tag=f"aq2_{i}")
            aq1s.append(a1)
            aq2s.append(a2)
            for a in (a1, a2):
                for blk in range(2):
                    nc.gpsimd.memset(a[:, blk, 0:PW + 1], 0.0)
                    mid = a[:, blk, PW + W + 1: PW + W + 1 + (H - 1) * PW]
                    mid3 = mid.rearrange("p (r c) -> p r c", c=PW)[:, :, 0:2]
                    nc.gpsimd.memset(mid3, 0.0)
                    nc.gpsimd.memset(a[:, blk, H * PW + W + 1: NPAD], 0.0)

        def quant_stage(src_ap, aq, blk, inv_col, bias_col, tmp_pool, rows, y0):
            """Exact replica of the reference fp32 op sequence:
            t=x*inv+b; relu; min(.,1)*3; round-to-nearest-even; cast."""
            t = tmp_pool.tile([128, rows * W], F32, tag="qtmp")
            nc.vector.tensor_scalar(t[:], src_ap, prm[:, inv_col:inv_col + 1],
                                    prm[:, bias_col:bias_col + 1], OP.mult, OP.add)
            nc.scalar.activation(t[:], t[:], mybir.ActivationFunctionType.Relu)
            nc.vector.tensor_scalar(t[:], t[:], 1.0, 3.0, OP.min, OP.mult)
            dst = aq[:, blk, (y0 + 1) * PW + 1: (y0 + 1) * PW + 1 + rows * PW]
            dst3 = dst.rearrange("p (r c) -> p r c", c=PW)[:, :, 0:W]
            nc.vector.tensor_scalar(dst3, t[:], MAGIC, MAGIC, OP.add, OP.subtract)

        def conv_tile(aq, w_sb, t, cb):
            ps = pspool.tile([128, TQ], F32, tag="ps")
            w4 = w_sb[:].rearrange("p (t j m) -> p t j m", t=9, j=2)
            for tap in range(9):
                ky, kx = divmod(tap, 3)
                lhsT = w4[:, tap, :, cb * 128:cb * 128 + 128]
                rhs = aq[:, :, t * TQ + ky * PW + kx: t * TQ + ky * PW + kx + TQ]
                nc.tensor.matmul(ps[:], lhsT, rhs,
                                 perf_mode=mybir.MatmulPerfMode.DoubleRow,
                                 start=(tap == 0), stop=(tap == 8))
            return ps

        # PE warmup on already-resident weight data during the stage-1(img0)
        # latency window, so the HAM clock gate reaches 8/8 early.
        if N_WARMUP:
            wu_ps = pspool.tile([128, TQ], F32, tag="ps")
            for i in range(N_WARMUP):
                nc.tensor.matmul(wu_ps[:], w1_sb[:, 0:128], w1_sb[:, 0:TQ],
                                 start=(i == 0), stop=(i == N_WARMUP - 1))

        x_sbs = [None] * N_IMG
        last_xdma = [None]

        def stage1_dma(img):
            """Issue image img's input DMA, serialized behind the previous
            image's so the startup-critical image gets full HBM bandwidth."""
            x_sb = xpool.tile([128, 2, NPIX], F32, tag="x", name=f"x_{img}")
            x_sbs[img] = x_sb
            first = None
            for y0, rr in CHUNKS:
                sl = slice(y0 * W, (y0 + rr) * W)
                inst = nc.sync.dma_start(x_sb[:, :, sl], xr[img][:, :, sl])
                if first is None and last_xdma[0] is not None:
                    tile.add_dep_helper(last_xdma[0].ins, inst.ins, sync=True,
                                        reason="serialize per-image input DMA")
                first = first if first is not None else inst
                last_xdma[0] = inst
            return x_sb

        def stage1_units(img):
            """Quant thunks for image img, one per (chunk, blk)."""
            aq1 = aq1s[img % 2]
            x_sb = x_sbs[img]

            def make(y0, rr, blk):
                def run():
                    sl = slice(y0 * W, (y0 + rr) * W)
                    quant_stage(x_sb[:, blk, sl], aq1, blk, 0 + blk, 2 + blk,
                                t1pool, rr, y0)
                return run
            return [make(y0, rr, blk) for (y0, rr) in CHUNKS for blk in range(2)]

        def conv1_tile(img, t, cb):
            aq1, aq2 = aq1s[img % 2], aq2s[img % 2]
            ps = conv_tile(aq1, w1_sb, t, cb)
            psv = ps[:].rearrange("p (r c) -> p r c", c=PW)[:, :, 0:W]
            quant_stage(psv, aq2, cb, 4 + cb, 6 + cb, t2pool, RT, t * RT)

        def conv2_tile(img, t, cb):
            aq2, x_sb = aq2s[img % 2], x_sbs[img]
            ps = conv_tile(aq2, w2_sb, t, cb)
            psv = ps[:].rearrange("p (r c) -> p r c", c=PW)[:, :, 0:W]
            res = x_sb[:, cb, t * RT * W: (t + 1) * RT * W]
            res3 = res.rearrange("p (r c) -> p r c", c=W)
            nc.vector.scalar_tensor_tensor(res3, psv, 1.0 / 9.0, res3,
                                           OP.mult, OP.add)
            nc.sync.dma_start(outr[img][:, cb, t * RT * W:
                                        (t + 1) * RT * W], res)

        def conv1_img(img, interleave=()):
            # Front-load the next image's stage-1 units into the first conv1
            # tiles so the tail of this block's DVE queue holds only the
            # epilogues the next conv block is waiting on.
            inter = list(interleave)
            for t in range(NT):
                for cb in range(2):
                    conv1_tile(img, t, cb)
                for _ in range(2):
                    if inter:
                        inter.pop(0)()
            for f in inter:
                f()

        def conv2_img(img):
            for t in range(NT):
                for cb in range(2):
                    conv2_tile(img, t, cb)

        # Software pipeline: stage 1 one image ahead (quant units spread
        # between conv1 tiles), conv2 one image behind.
        stage1_dma(0)
        for f in stage1_units(0):
            f()
        if N_IMG > 1:
            stage1_dma(1)
            conv1_img(0, interleave=stage1_units(1))
        else:
            conv1_img(0)
        for img in range(1, N_IMG):
            nxt = ()
            if img + 1 < N_IMG:
                stage1_dma(img + 1)
                nxt = stage1_units(img + 1)
            conv1_img(img, interleave=nxt)
            conv2_img(img - 1)
        conv2_img(N_IMG - 1)

    nc.compile()
    return nc


def _host_prep(w1, w2, g1, b1, m1, v1, g2, b2, m2, v2):
    """BN folds + DoReFa weight quantization, replicating the reference's
    fp32 op sequence exactly (jax CPU), then weight layout transforms."""
    import jax
    import jax.numpy as jnp
    import ml_dtypes

    cpu = jax.local_devices(backend="cpu")[0]
    with jax.default_device(cpu):
        eps = jnp.float32(1e-5)
        inv1 = g1 / jnp.sqrt(v1 + eps)
        bias1 = b1 - m1 * inv1
        inv2 = g2 / jnp.sqrt(v2 + eps)
        bias2 = b2 - m2 * inv2
        inv2_9 = inv2 / np.float32(9.0)

        def wq3(w):
            wt = jnp.tanh(w)
            wn = wt / (2.0 * jnp.max(jnp.abs(wt))) + 0.5
            return 2.0 * jnp.round(wn * 3.0) - 3.0   # exact ints {-3,-1,1,3}

        wq1 = np.asarray(wq3(jnp.asarray(w1)), dtype=np.float32)
        wq2 = np.asarray(wq3(jnp.asarray(w2)), dtype=np.float32)
        inv1, bias1, inv2_9, bias2 = (np.asarray(a, dtype=np.float32)
                                      for a in (inv1, bias1, inv2_9, bias2))

    def wlayout(wq):
        # [cout, cin, ky, kx] -> [k(128), tap(9), blk(2), cout(256)]
        a = wq.reshape(256, 2, 128, 9)                     # cout, blk, k, tap
        return np.ascontiguousarray(np.transpose(a, (2, 3, 1, 0))
                                    .reshape(128, 4608)
                                    ).astype(ml_dtypes.float8_e4m3)

    prm = np.zeros((128, 8), np.float32)
    for col, v in enumerate((inv1, bias1)):
        prm[:, 2 * col] = v[0:128]
        prm[:, 2 * col + 1] = v[128:256]
    for col, v in enumerate((inv2_9, bias2)):
        prm[:, 4 + 2 * col] = v[0:128]
        prm[:, 4 + 2 * col + 1] = v[128:256]

    return {"w1t": wlayout(wq1), "w2t": wlayout(wq2), "prm": prm}


def kernel(x, w1, w2, g1, b1, m1, v1, g2, b2, m2, v2):
    global LAST_EXEC_NS
    x = np.asarray(x, dtype=np.float32)

    if "nc" not in _CACHED:
        _CACHED["nc"] = _build()
    nc = _CACHED["nc"]

    shared = _host_prep(w1, w2, g1, b1, m1, v1, g2, b2, m2, v2)
    in_maps = []
    for c in range(N_CORES):
        m = dict(shared)
        m["x"] = x[N_IMG * c:N_IMG * (c + 1)]
        in_maps.append(m)

    trace = bool(int(os.environ.get("BASS_TRACE", "0")))
    res = run_bass_kernel_spmd(nc, in_maps, core_ids=list(range(N_CORES)),
                               trace=trace)
    LAST_EXEC_NS = res.exec_time_ns
    return np.concatenate([res.results[c]["out"] for c in range(N_CORES)],
                          axis=0)
